# revision 30
# baseline (speedup 1.0000x reference)
"""GQA attention kernel for Trainium2, 8 NeuronCores.

Sharding: core i handles (batch b = i//4, kv-group g = i%4) -> 4 query heads.
Each core computes its group's partial out-projection in bf16; host sums the
4 partials per batch element (the "all-reduce after out_proj").

Dataflow is fully transposed on-device (head_dim on SBUF partitions):
  qT/kT = W.T-stationary projections of xT; RoPE rotate-half done as two
  64-partition SBUF->SBUF DMA copies on the idle SP queue (sign folded
  into the sin tables), so PE carries no rotation matmuls (the rope
  outputs are consumed only in phase 2, so DMA latency is free);
  RMS-norm factors via Pool
  partition_all_reduce of the squared raws (result broadcast to all
  partitions), ACT sqrt + DVE reciprocal on the [128, blk] tile -- no PE
  ones-matmuls and no separate partition_broadcast.  The norm factor is
  per-column, so it commutes through rope and is applied once at the end
  for BOTH q and k.  Scores S^T = kT-tile.T @ qT ([sk,sq] layout), softmax
  denominator via Pool partition_all_reduce of the accumulated exp sums,
  ctx^T accumulated over sk tiles with v stationary and normalized by a
  single DVE multiply straight out of PSUM, out = ctxT-tile-stationary @ Wo
  chunks accumulated over heads.

Performance structure (engine streams execute in-order; emission order is
the schedule):
  - input DMAs split across SP/ACT HWDGE queues + Pool SWDGE in exact
    compute-consumption order (xT chunk c arrives just before the K
    projection consumes it; wq chunks before the first Q unit needs them)
  - two ACT table loads only: sqrt set for the projection phase, exp set
    for the attention phase; all phase-1 ACT ops precede the first exp
  - norm/rope chains for K blocks and Q (h,Bb) units share a staged
    software pipeline (prep at slot-1, norm+rope at slot-2); the PE stream
    is pure projection matmuls and never waits on a chain
  - attention runs on sk-tile pairs: two score matmuls land in one 2-bank
    PSUM tile, one scale-free exp covers clean pairs (post-norm scores are
    O(sqrt(dh)), exp is safe without max subtraction); causal masks
    applied on Pool via a triangular multiply
  - ctx matmuls one pair behind scores; V projection tiles and out-proj
    row stripes dropped into B0/B1 and B2/B3 units as PE filler;
    denominator chains deferred one unit (pre_pe)
  - output written bf16, one DMA per 128-row stripe on SP; the final
    stripe issues per-chunk DMAs with a small 128-column tail chunk
"""

import sys

sys.path.insert(0, "/opt/trn_rl_repo")

import numpy as np
import ml_dtypes

import concourse.bass as bass
import concourse.tile as tile
from concourse import bacc
from concourse import bass_isa
from concourse import mybir
from concourse.bass import ts
from concourse.bass_utils import run_bass_kernel_spmd

BF16 = ml_dtypes.bfloat16

B = 2           # batch
S = 2048        # sequence
D = 2048        # model dim
HL = 4          # heads per core (local)
DH = 128        # head dim
NC_ = 16        # d-chunks of 128
NT = 16         # seq tiles of 128
NB = 4          # seq blocks of 512
BLK = 512
EPS = 1e-6

_CACHED_NC = None

def build_nc():
    dt = mybir.dt
    nc = bacc.Bacc()

    xT = nc.declare_dram_parameter("xT", [NC_, 128, S], dt.bfloat16, isOutput=False)
    wq = nc.declare_dram_parameter("wq", [NC_, 128, HL * DH], dt.bfloat16, isOutput=False)
    wk = nc.declare_dram_parameter("wk", [128, NC_ * DH], dt.bfloat16, isOutput=False)
    wv = nc.declare_dram_parameter("wv", [128, NC_ * DH], dt.bfloat16, isOutput=False)
    wo = nc.declare_dram_parameter("wo", [HL, 128, D], dt.bfloat16, isOutput=False)
    cosq = nc.declare_dram_parameter("cosq", [128, S], dt.bfloat16, isOutput=False)
    sinq = nc.declare_dram_parameter("sinq", [128, S], dt.bfloat16, isOutput=False)
    cosk = nc.declare_dram_parameter("cosk", [128, S], dt.bfloat16, isOutput=False)
    sink = nc.declare_dram_parameter("sink", [128, S], dt.bfloat16, isOutput=False)
    tri = nc.declare_dram_parameter("tri", [128, 128], dt.bfloat16, isOutput=False)
    out = nc.declare_dram_parameter("out", [S, D], dt.bfloat16, isOutput=True)

    with tile.TileContext(nc) as tc:
        with (
            tc.tile_pool(name="singles", bufs=1) as singles,
            tc.tile_pool(name="work", bufs=2) as work,
            tc.tile_pool(name="expp", bufs=4) as expp,
            tc.tile_pool(name="accp", bufs=2) as accp,
            tc.tile_pool(name="outp", bufs=2) as outp,
            tc.tile_pool(name="ps2", bufs=2, space="PSUM") as ps2,
            tc.tile_pool(name="pmm", bufs=3, space="PSUM") as pmm,
            tc.tile_pool(name="pctx", bufs=1, space="PSUM") as pctx,
        ):
            # ---- resident SBUF tensors ----
            xT_s = singles.tile([128, NC_, S], dt.bfloat16)
            wq_s = singles.tile([128, NC_, HL * DH], dt.bfloat16)
            wk_s = singles.tile([128, NC_ * DH], dt.bfloat16)
            wv_s = singles.tile([128, NC_ * DH], dt.bfloat16)
            wo_s = singles.tile([128, HL, D], dt.bfloat16)
            cosq_s = singles.tile([128, S], dt.bfloat16)
            sinq_s = singles.tile([128, S], dt.bfloat16)
            cosk_s = singles.tile([128, S], dt.bfloat16)
            sink_s = singles.tile([128, S], dt.bfloat16)
            tri_s = singles.tile([128, 128], dt.bfloat16)
            eps_s = singles.tile([128, 1], dt.float32)
            qT_s = singles.tile([128, HL, S], dt.bfloat16)
            kT_s = singles.tile([128, S], dt.bfloat16)
            v_s = singles.tile([128, NT, DH], dt.bfloat16)
            ctxT_s = singles.tile([128, HL, S], dt.bfloat16)

            nc.vector.memset(eps_s, EPS)

            # ---- input DMAs, in compute-consumption order per queue ----
            # Early xT chunks go out in 1024-col halves to halve the
            # per-chunk delivery latency while the K projection chases them;
            # wk is split fine-grained the same way.  Queue assignment keeps
            # each queue's k-th transfer ahead of PE's consumption of it.
            HB = 2 * BLK
            # gpsimd SWDGE: wk chunks 0-3 first (tiny), odd-late xT chunks
            nc.gpsimd.dma_start(out=wk_s[:, :DH], in_=wk[:, :DH])
            nc.gpsimd.dma_start(out=wk_s[:, DH : 2 * DH], in_=wk[:, DH : 2 * DH])
            nc.gpsimd.dma_start(out=wk_s[:, 2 * DH : 4 * DH], in_=wk[:, 2 * DH : 4 * DH])
            # sync (SP): chunk 0 split 512/1536 so block 0 lands first
            nc.sync.dma_start(out=xT_s[:, 0, :BLK], in_=xT[0][:, :BLK])
            nc.sync.dma_start(out=xT_s[:, 0, BLK:], in_=xT[0][:, BLK:])
            # scalar (ACT): xT1 halves first (the queue sits behind the
            # 1.3us act-table load anyway), xT3 halves, rest of wk
            nc.scalar.dma_start(out=xT_s[:, 1, :HB], in_=xT[1][:, :HB])
            nc.scalar.dma_start(out=xT_s[:, 1, HB:], in_=xT[1][:, HB:])
            nc.sync.dma_start(out=xT_s[:, 2, :HB], in_=xT[2][:, :HB])
            nc.sync.dma_start(out=xT_s[:, 2, HB:], in_=xT[2][:, HB:])
            nc.scalar.dma_start(out=xT_s[:, 3, :HB], in_=xT[3][:, :HB])
            nc.scalar.dma_start(out=xT_s[:, 3, HB:], in_=xT[3][:, HB:])
            nc.sync.dma_start(out=xT_s[:, 4, :HB], in_=xT[4][:, :HB])
            nc.sync.dma_start(out=xT_s[:, 4, HB:], in_=xT[4][:, HB:])
            nc.scalar.dma_start(out=wk_s[:, 4 * DH : 8 * DH], in_=wk[:, 4 * DH : 8 * DH])
            nc.scalar.dma_start(out=wk_s[:, 8 * DH :], in_=wk[:, 8 * DH :])
            nc.sync.dma_start(out=xT_s[:, 6, :HB], in_=xT[6][:, :HB])
            nc.sync.dma_start(out=xT_s[:, 6, HB:], in_=xT[6][:, HB:])
            for c in (8, 10, 12, 14):
                nc.sync.dma_start(out=xT_s[:, c, :], in_=xT[c])
            for c in (5, 7, 9, 11, 13, 15):
                nc.gpsimd.dma_start(out=xT_s[:, c, :], in_=xT[c])
            for c in range(6):
                nc.sync.dma_start(out=wq_s[:, c, :], in_=wq[c])
            for c in range(6, 11):
                nc.scalar.dma_start(out=wq_s[:, c, :], in_=wq[c])
            for c in range(11, NC_):
                nc.gpsimd.dma_start(out=wq_s[:, c, :], in_=wq[c])
            nc.scalar.dma_start(out=cosk_s[:], in_=cosk[:])
            nc.scalar.dma_start(out=sink_s[:], in_=sink[:])
            nc.scalar.dma_start(out=cosq_s[:], in_=cosq[:])
            nc.scalar.dma_start(out=sinq_s[:], in_=sinq[:])
            nc.gpsimd.dma_start(out=wv_s[:], in_=wv[:])
            nc.gpsimd.dma_start(out=tri_s[:], in_=tri[:])
            for h in range(HL):
                nc.gpsimd.dma_start(out=wo_s[:, h, :], in_=wo[h])

            Exp = mybir.ActivationFunctionType.Exp
            Sqrt = mybir.ActivationFunctionType.Sqrt
            Copy = mybir.ActivationFunctionType.Copy
            RADD = bass_isa.ReduceOp.add

            # ---- PE warm-up: dummy matmuls on a zeroed tile while the
            # first input DMAs are in flight, so the p-state ramp is burned
            # before real work arrives ----
            warm = work.tile([128, BLK], dt.bfloat16, tag="warm", bufs=1)
            weps = work.tile([128, 1], dt.bfloat16, tag="weps", bufs=1)
            nc.vector.memset(weps, 0.0)
            nc.vector.memset(warm, 0.0)
            ps_w = pmm.tile([1, BLK], dt.float32, tag="mm", name="ps_warm")
            for _ in range(1):
                nc.tensor.matmul(ps_w, weps, warm, start=True, stop=True)

            # ================= phase 1: projections (ACT: sqrt set) =========
            # K projection, chunk-outer into two 2-bank pair tiles: each
            # arriving xT chunk feeds 4 matmuls, PE chases the input DMAs.
            ps_kp = [
                ps2.tile([128, 2 * BLK], dt.float32, tag="s2", name=f"ps_kp{i}")
                for i in range(2)
            ]
            for c in range(NC_):
                for Bb in range(NB):
                    nc.tensor.matmul(
                        ps_kp[Bb // 2][:, (Bb % 2) * BLK : (Bb % 2) * BLK + BLK],
                        wk_s[:, ts(c, DH)], xT_s[:, c, ts(Bb, BLK)],
                        start=(c == 0), stop=(c == NC_ - 1),
                    )

            # Unified norm/rope pipeline units: 4 K blocks + 16 Q (h,Bb).
            # Stages: proj (PE, q only) -> prep (ACT raw copy [q], DVE
            # square, Pool rotate-half via 2 stream_shuffles) -> norm (Pool
            # all-reduce, ACT sqrt, DVE reciprocal) -> rope combine (DVE
            # t2=raw*cos, t1=rot*sin_signed, t12=t1+t2, dst=t12*rn_b).
            state = {}

            def k_raw(Bb):
                kraw = work.tile(
                    [128, BLK], dt.bfloat16, tag=f"kraw{Bb}", bufs=1,
                    name=f"kraw{Bb}",
                )
                nc.scalar.activation(
                    kraw, ps_kp[Bb // 2][:, (Bb % 2) * BLK : (Bb % 2) * BLK + BLK],
                    Copy,
                )
                state[("k", Bb)] = {"raw": kraw}

            def q_proj(i):
                Bb, h = divmod(i, HL)
                ps_q = pmm.tile([128, BLK], dt.float32, tag="mm")
                for c in range(NC_):
                    nc.tensor.matmul(
                        ps_q, wq_s[:, c, ts(h, DH)], xT_s[:, c, ts(Bb, BLK)],
                        start=(c == 0), stop=(c == NC_ - 1),
                    )
                state[("q", i)] = {"ps": ps_q}

            def u_prep(u):
                """raw copy (q only; k copies are pre-loop) + square + rot."""
                st = state[u]
                if u[0] == "q":
                    qraw = work.tile([128, BLK], dt.bfloat16, tag="qraw", bufs=3)
                    nc.scalar.activation(qraw, st.pop("ps"), Copy)
                    st["raw"] = qraw
                rot = work.tile([128, BLK], dt.bfloat16, tag="rot", bufs=2)
                nc.sync.dma_start(out=rot[:64, :], in_=st["raw"][64:, :])
                nc.sync.dma_start(out=rot[64:, :], in_=st["raw"][:64, :])
                st["rot"] = rot
                sq = work.tile([128, BLK], dt.bfloat16, tag="sq", bufs=2)
                nc.vector.tensor_mul(sq, st["raw"], st["raw"])
                st["sq"] = sq

            def u_norm(u):
                st = state[u]
                ss = work.tile([128, BLK], dt.float32, tag="ss", bufs=2)
                nc.gpsimd.partition_all_reduce(ss, st.pop("sq"), 128, RADD)
                rn_b = work.tile([128, BLK], dt.bfloat16, tag="rnb", bufs=2)
                nc.scalar.activation(rn_b, ss, Sqrt, scale=1.0 / DH, bias=eps_s)
                with nc.allow_low_precision(reason="rsqrt norm factor, bf16 ok"):
                    nc.vector.reciprocal(rn_b, rn_b)
                st["rn_b"] = rn_b

            def u_rope(u):
                st = state[u]
                if u[0] == "q":
                    Bb, h = divmod(u[1], HL)
                    cos_s, sin_s = cosq_s, sinq_s
                    dst = qT_s[:, h, ts(Bb, BLK)]
                else:
                    Bb = u[1]
                    cos_s, sin_s = cosk_s, sink_s
                    dst = kT_s[:, ts(Bb, BLK)]
                t2 = work.tile([128, BLK], dt.bfloat16, tag="t2", bufs=1)
                nc.vector.tensor_mul(t2, st["raw"], cos_s[:, ts(Bb, BLK)])
                t1 = work.tile([128, BLK], dt.bfloat16, tag="t1", bufs=1)
                nc.vector.tensor_mul(t1, st.pop("rot"), sin_s[:, ts(Bb, BLK)])
                t12 = work.tile([128, BLK], dt.bfloat16, tag="t12", bufs=1)
                nc.vector.tensor_add(t12, t2, t1)
                nc.vector.tensor_mul(dst, t12, st.pop("rn_b"))
                st.pop("raw", None)

            for Bb in range(NB):
                k_raw(Bb)
            units = []
            for j in range(NB):
                units.append(("k", j))
                units.append(("q", j))
            for i in range(NB, NB * HL):
                units.append(("q", i))

            for idx, u in enumerate(units):
                if u[0] == "q":
                    q_proj(u[1])
                if idx >= 1:
                    u_prep(units[idx - 1])
                if idx >= 2:
                    u_norm(units[idx - 2])
                    u_rope(units[idx - 2])
            u_prep(units[-1])
            u_norm(units[-2])
            u_rope(units[-2])
            u_norm(units[-1])
            u_rope(units[-1])

            # ---- V projection tiles: emitters used as phase-2 PE filler ----
            def make_vproj(t):
                def emit():
                    ps_v = pmm.tile([128, DH], dt.float32, tag="mm", name="ps_v")
                    for c in range(NC_):
                        nc.tensor.matmul(
                            ps_v, xT_s[:, c, ts(t, 128)], wv_s[:, ts(c, DH)],
                            start=(c == 0), stop=(c == NC_ - 1),
                        )
                    nc.vector.tensor_copy(v_s[:, t, :], ps_v)
                return emit

            # ================= phase 2: attention + out-proj (ACT: exp) =====
            osb_n = 0

            def attention(h, Bb, pre_pe=(), fillers=(), late_den=None):
                """sk-tile pairs: 2 score matmuls into one 2-bank PSUM tile,
                one scale-free exp per clean pair (per-half for staircase
                pairs), ctx matmuls one pair behind. pre_pe lands after pair
                0's matmuls; fillers dropped one per pair from pair 1 on,
                leftovers before the final ctx pair."""
                nonlocal osb_n
                ntile = 4 * Bb + 4
                npair = ntile // 2
                ps_ctx = pctx.tile([128, BLK], dt.float32, tag="ctx")
                acc = accp.tile([128, 2 * BLK], dt.bfloat16, tag="sumacc")
                exps = {}
                j0s = {}
                fill = list(fillers)
                p_drop = 2 if npair > 2 else 1
                fdrop = max(p_drop, npair // 2)

                def emit_ctx_pair(p):
                    for half in (0, 1):
                        t = 2 * p + half
                        nc.tensor.matmul(
                            ps_ctx[:, j0s[t]:], v_s[:, t, :],
                            exps[p][:, half * BLK + j0s[t] : (half + 1) * BLK],
                            start=(t == 0), stop=(t == ntile - 1),
                        )

                for p in range(npair):
                    ta, tb = 2 * p, 2 * p + 1
                    ja = max(0, ta * 128 - Bb * BLK)
                    jb = max(0, tb * 128 - Bb * BLK)
                    diag_a, diag_b = ta * 128 >= Bb * BLK, tb * 128 >= Bb * BLK
                    ps_S = ps2.tile([128, 2 * BLK], dt.float32, tag="s2", name="ps_S")
                    single = ja == 0 and not diag_b
                    nc.tensor.matmul(
                        ps_S[:, ja:BLK],
                        kT_s[:, ts(ta, 128)],
                        qT_s[:, h, Bb * BLK + ja : (Bb + 1) * BLK],
                        start=True, stop=True,
                    )
                    nc.tensor.matmul(
                        ps_S[:, BLK + jb : 2 * BLK],
                        kT_s[:, ts(tb, 128)],
                        qT_s[:, h, Bb * BLK + jb : (Bb + 1) * BLK],
                        start=True, stop=True,
                    )
                    if p == p_drop:
                        for f in pre_pe:
                            f()
                    if p >= fdrop and fill:
                        fill.pop(0)()
                    # exp first (post-norm scores are O(sqrt(dh)), so the
                    # masked region stays finite); zero masked columns on
                    # Pool afterwards (GPSIMD cannot touch PSUM, but expS
                    # lives in SBUF)
                    expS = expp.tile([128, 2 * BLK], dt.bfloat16, tag="exp")
                    if single:
                        nc.scalar.activation(expS[:], ps_S[:], Exp)
                    else:
                        nc.scalar.activation(expS[:, ja:BLK], ps_S[:, ja:BLK], Exp)
                        nc.scalar.activation(
                            expS[:, BLK + jb :], ps_S[:, BLK + jb :], Exp
                        )
                    if diag_a:
                        nc.gpsimd.tensor_mul(
                            expS[:, ja : ja + 128], expS[:, ja : ja + 128], tri_s
                        )
                    if diag_b:
                        nc.gpsimd.tensor_mul(
                            expS[:, BLK + jb : BLK + jb + 128],
                            expS[:, BLK + jb : BLK + jb + 128], tri_s,
                        )
                    if single:
                        if p == 0:
                            nc.vector.tensor_copy(acc, expS)
                        elif Bb == 3 and p <= 2:
                            # B3 units are DVE-tight; Pool has slack
                            nc.gpsimd.tensor_add(acc, acc, expS)
                        else:
                            nc.vector.tensor_add(acc, acc, expS)
                    elif p == 0:
                        # B0's first pair: init acc per-half, zero the gap
                        nc.vector.tensor_copy(acc[:, :BLK], expS[:, :BLK])
                        nc.gpsimd.memset(acc[:, BLK : BLK + jb], 0.0)
                        nc.vector.tensor_copy(
                            acc[:, BLK + jb :], expS[:, BLK + jb :]
                        )
                    else:
                        nc.vector.tensor_add(
                            acc[:, ja:BLK], acc[:, ja:BLK], expS[:, ja:BLK]
                        )
                        nc.vector.tensor_add(
                            acc[:, BLK + jb :], acc[:, BLK + jb :],
                            expS[:, BLK + jb :],
                        )
                    exps[p] = expS
                    j0s[ta] = ja
                    j0s[tb] = jb
                    if p >= 2:
                        emit_ctx_pair(p - 2)
                if late_den is not None:
                    late_den(acc)
                for f in fill:
                    f()
                emit_ctx_pair(npair - 2)
                emit_ctx_pair(npair - 1)
                return ps_ctx, acc

            def make_den_sum(acc, holder):
                def emit():
                    acc2 = work.tile([128, BLK], dt.bfloat16, tag="acc2", bufs=1)
                    nc.gpsimd.tensor_add(acc2, acc[:, :BLK], acc[:, BLK:])
                    den = work.tile([128, BLK], dt.float32, tag="den", bufs=1)
                    nc.gpsimd.partition_all_reduce(den, acc2, 128, RADD)
                    recip_b = work.tile([128, BLK], dt.bfloat16, tag="recipb")
                    with nc.allow_low_precision(reason="softmax denom, bf16 ok"):
                        nc.vector.reciprocal(recip_b, den)
                    holder["r"] = recip_b
                return emit

            def make_den_mul(h, Bb, ps_ctx, holder):
                def emit():
                    nc.vector.tensor_mul(
                        ctxT_s[:, h, ts(Bb, BLK)], ps_ctx, holder["r"]
                    )
                return emit

            def make_outproj(st, split=False, dve_only=False, act_only=False,
                             fine_tail=False):
                def emit():
                    nonlocal osb_n
                    osb = outp.tile([128, D], dt.bfloat16, tag="osb")
                    chunks = [(oc * BLK, BLK) for oc in range(4)]
                    if fine_tail:
                        chunks = chunks[:3] + [(3 * BLK, 448), (3 * BLK + 448, 64)]
                    for j0, w in chunks:
                        ps_o = pmm.tile([128, BLK], dt.float32, tag="mm", name="ps_o")
                        for h in range(HL):
                            nc.tensor.matmul(
                                ps_o[:, :w], ctxT_s[:, h, ts(st, 128)],
                                wo_s[:, h, j0 : j0 + w],
                                start=(h == 0), stop=(h == HL - 1),
                            )
                        if act_only:
                            nc.scalar.activation(osb[:, j0 : j0 + w], ps_o[:, :w], Copy)
                        elif dve_only or osb_n % 2 == 0:
                            nc.vector.tensor_copy(osb[:, j0 : j0 + w], ps_o[:, :w])
                        else:
                            nc.scalar.activation(osb[:, j0 : j0 + w], ps_o[:, :w], Copy)
                        osb_n += 1
                        if split:
                            nc.sync.dma_start(
                                out=out[ts(st, 128), j0 : j0 + w],
                                in_=osb[:, j0 : j0 + w],
                            )
                    if not split:
                        nc.sync.dma_start(out=out[ts(st, 128), :], in_=osb)
                return emit

            # filler schedule: V tiles into B0/B1 units, out-proj stripes
            # one block behind, last blocks' stripes in the tail
            order = [(Bb, h) for Bb in range(NB) for h in range(HL)]
            fillers = {
                (0, 0): [make_vproj(0), make_vproj(1), make_vproj(2), make_vproj(3)],
                (0, 1): [make_vproj(4), make_vproj(5)],
                (0, 2): [make_vproj(6), make_vproj(7)],
                (0, 3): [make_vproj(8), make_vproj(9)],
                (1, 0): [make_vproj(10), make_vproj(11)],
                (1, 1): [make_vproj(12), make_vproj(13)],
                (1, 2): [make_vproj(14), make_vproj(15)],
                (1, 3): [make_outproj(0)],
                (2, 0): [make_outproj(1)],
                (2, 1): [make_outproj(2)],
                (2, 2): [make_outproj(3)],
                (2, 3): [make_outproj(4)],
                (3, 0): [make_outproj(5, dve_only=True), make_outproj(6, dve_only=True)],
                (3, 1): [make_outproj(7, dve_only=True), make_outproj(8, dve_only=True)],
                (3, 2): [make_outproj(9, dve_only=True)],
                (3, 3): [],
            }

            # last unit: the denominator sum (acc halves + all-reduce +
            # reciprocal) is emitted inside the unit right after its last
            # exp-sum so the chain overlaps the final ctx pairs; the ctxT
            # multiply lands right after, before the tail's stripe matmuls
            # need it
            last = order[-1]
            pend_den = []
            for Bb, h in order:
                hold = {}
                late = None
                if (Bb, h) == last:
                    late = lambda a, hd=hold: make_den_sum(a, hd)()
                ps_ctx, acc = attention(
                    h, Bb, pre_pe=pend_den, fillers=fillers[(Bb, h)],
                    late_den=late,
                )
                if (Bb, h) == last:
                    pend_den = [make_den_mul(h, Bb, ps_ctx, hold)]
                else:
                    pend_den = [
                        make_den_sum(acc, hold), make_den_mul(h, Bb, ps_ctx, hold)
                    ]
            for f in pend_den:
                f()
            for st2 in (10, 11, 12, 13, 14):
                make_outproj(st2)()
            make_outproj(15, split=True, fine_tail=True)()
    nc.finalize()
    return nc


def _host_inputs(x, cos, sin, Wq, Wk, Wv, Wo, qn_w, kn_w):
    """Build the 8 per-core input maps (host-side sharding + layout prep)."""
    scale = DH ** -0.5
    qn_rot = np.concatenate([qn_w[64:], qn_w[:64]])
    kn_rot = np.concatenate([kn_w[64:], kn_w[:64]])
    # rotate-half sign: rows 0..63 of the rotated tensor carry -x2, so the
    # sin tables are negated on those partitions (rotation itself is an
    # unsigned 64-partition swap on Pool)
    sgn = np.ones((128, 1), dtype=np.float32)
    sgn[:64] = -1.0
    cosq = (cos.T * qn_w[:, None] * scale).astype(BF16)
    sinq = (sin.T * qn_rot[:, None] * scale * sgn).astype(BF16)
    cosk = (cos.T * kn_w[:, None]).astype(BF16)
    sink = (sin.T * kn_rot[:, None] * sgn).astype(BF16)
    ii = np.arange(128)
    tri = np.where(ii[None, :] < ii[:, None], 0.0, 1.0).astype(BF16)

    def cmajor(w):  # [2048, 128] -> [128, 16*128] (chunk-major free dim)
        return np.ascontiguousarray(
            w.reshape(NC_, 128, DH).transpose(1, 0, 2).reshape(128, NC_ * DH)
        ).astype(BF16)

    in_maps = []
    for core in range(8):
        b, g = core // 4, core % 4
        xTb = np.ascontiguousarray(x[b].T).astype(BF16).reshape(NC_, 128, S)
        in_maps.append({
            "xT": xTb,
            "wq": Wq[:, g * 512 : (g + 1) * 512].astype(BF16).reshape(NC_, 128, 512),
            "wk": cmajor(Wk[:, g * 128 : (g + 1) * 128]),
            "wv": cmajor(Wv[:, g * 128 : (g + 1) * 128]),
            "wo": Wo[g * 512 : (g + 1) * 512, :].astype(BF16).reshape(HL, 128, D),
            "cosq": cosq, "sinq": sinq, "cosk": cosk, "sink": sink,
            "tri": tri,
        })
    return in_maps


def kernel(x, mask, cos, sin, Wq, Wk, Wv, Wo, qn_w, kn_w, _trace=False):
    global _CACHED_NC
    x = np.asarray(x, dtype=np.float32)
    cos = np.asarray(cos, dtype=np.float32)
    sin = np.asarray(sin, dtype=np.float32)
    Wq = np.asarray(Wq, dtype=np.float32)
    Wk = np.asarray(Wk, dtype=np.float32)
    Wv = np.asarray(Wv, dtype=np.float32)
    Wo = np.asarray(Wo, dtype=np.float32)
    qn_w = np.asarray(qn_w, dtype=np.float32)
    kn_w = np.asarray(kn_w, dtype=np.float32)

    if _CACHED_NC is None:
        _CACHED_NC = build_nc()
    nc = _CACHED_NC
    in_maps = _host_inputs(x, cos, sin, Wq, Wk, Wv, Wo, qn_w, kn_w)
    res = run_bass_kernel_spmd(nc, in_maps, list(range(8)), trace=_trace)
    out = np.zeros((B, S, D), dtype=np.float32)
    for core in range(8):
        b = core // 4
        out[b] += np.asarray(res.results[core]["out"], dtype=np.float32)
    if _trace:
        return out, res
    return out


# revision 31
# speedup vs baseline: 1.0050x; 1.0050x over previous
"""GQA attention kernel for Trainium2, 8 NeuronCores.

Sharding: core i handles (batch b = i//4, kv-group g = i%4) -> 4 query heads.
Each core computes its group's partial out-projection in bf16; host sums the
4 partials per batch element (the "all-reduce after out_proj").

Dataflow is fully transposed on-device (head_dim on SBUF partitions):
  qT/kT = W.T-stationary projections of xT; RoPE rotate-half done as two
  64-partition SBUF->SBUF DMA copies on the idle SP queue (sign folded
  into the sin tables), so PE carries no rotation matmuls (the rope
  outputs are consumed only in phase 2, so DMA latency is free);
  RMS-norm factors via Pool
  partition_all_reduce of the squared raws (result broadcast to all
  partitions), ACT sqrt + DVE reciprocal on the [128, blk] tile -- no PE
  ones-matmuls and no separate partition_broadcast.  The norm factor is
  per-column, so it commutes through rope and is applied once at the end
  for BOTH q and k.  Scores S^T = kT-tile.T @ qT ([sk,sq] layout), softmax
  denominator via Pool partition_all_reduce of the accumulated exp sums,
  ctx^T accumulated over sk tiles with v stationary and normalized by a
  single DVE multiply straight out of PSUM, out = ctxT-tile-stationary @ Wo
  chunks accumulated over heads.

Performance structure (engine streams execute in-order; emission order is
the schedule):
  - input DMAs split across SP/ACT HWDGE queues + Pool SWDGE in exact
    compute-consumption order (xT chunk c arrives just before the K
    projection consumes it; wq chunks before the first Q unit needs them)
  - two ACT table loads only: sqrt set for the projection phase, exp set
    for the attention phase; all phase-1 ACT ops precede the first exp
  - norm/rope chains for K blocks and Q (h,Bb) units share a staged
    software pipeline (prep at slot-1, norm+rope at slot-2); the PE stream
    is pure projection matmuls and never waits on a chain
  - attention runs on sk-tile pairs: two score matmuls land in one 2-bank
    PSUM tile, one scale-free exp covers clean pairs (post-norm scores are
    O(sqrt(dh)), exp is safe without max subtraction); causal masks
    applied on Pool via a triangular multiply
  - ctx matmuls one pair behind scores; V projection tiles and out-proj
    row stripes dropped into B0/B1 and B2/B3 units as PE filler;
    denominator chains deferred one unit (pre_pe)
  - output written bf16, one DMA per 128-row stripe on SP; the final
    stripe issues per-chunk DMAs with a small 128-column tail chunk
"""

import sys

sys.path.insert(0, "/opt/trn_rl_repo")

import numpy as np
import ml_dtypes

import concourse.bass as bass
import concourse.tile as tile
from concourse import bacc
from concourse import bass_isa
from concourse import mybir
from concourse.bass import ts
from concourse.bass_utils import run_bass_kernel_spmd

BF16 = ml_dtypes.bfloat16

B = 2           # batch
S = 2048        # sequence
D = 2048        # model dim
HL = 4          # heads per core (local)
DH = 128        # head dim
NC_ = 16        # d-chunks of 128
NT = 16         # seq tiles of 128
NB = 4          # seq blocks of 512
BLK = 512
EPS = 1e-6

_CACHED_NC = None

def build_nc():
    dt = mybir.dt
    nc = bacc.Bacc()

    xT = nc.declare_dram_parameter("xT", [NC_, 128, S], dt.bfloat16, isOutput=False)
    wq = nc.declare_dram_parameter("wq", [NC_, 128, HL * DH], dt.bfloat16, isOutput=False)
    wk = nc.declare_dram_parameter("wk", [128, NC_ * DH], dt.bfloat16, isOutput=False)
    wv = nc.declare_dram_parameter("wv", [128, NC_ * DH], dt.bfloat16, isOutput=False)
    wo = nc.declare_dram_parameter("wo", [HL, 128, D], dt.bfloat16, isOutput=False)
    cosq = nc.declare_dram_parameter("cosq", [128, S], dt.bfloat16, isOutput=False)
    sinq = nc.declare_dram_parameter("sinq", [128, S], dt.bfloat16, isOutput=False)
    cosk = nc.declare_dram_parameter("cosk", [128, S], dt.bfloat16, isOutput=False)
    sink = nc.declare_dram_parameter("sink", [128, S], dt.bfloat16, isOutput=False)
    tri = nc.declare_dram_parameter("tri", [128, 128], dt.bfloat16, isOutput=False)
    out = nc.declare_dram_parameter("out", [S, D], dt.bfloat16, isOutput=True)

    with tile.TileContext(nc) as tc:
        with (
            tc.tile_pool(name="singles", bufs=1) as singles,
            tc.tile_pool(name="work", bufs=2) as work,
            tc.tile_pool(name="expp", bufs=4) as expp,
            tc.tile_pool(name="accp", bufs=2) as accp,
            tc.tile_pool(name="outp", bufs=2) as outp,
            tc.tile_pool(name="ps2", bufs=2, space="PSUM") as ps2,
            tc.tile_pool(name="pmm", bufs=3, space="PSUM") as pmm,
            tc.tile_pool(name="pctx", bufs=1, space="PSUM") as pctx,
        ):
            # ---- resident SBUF tensors ----
            xT_s = singles.tile([128, NC_, S], dt.bfloat16)
            wq_s = singles.tile([128, NC_, HL * DH], dt.bfloat16)
            wk_s = singles.tile([128, NC_ * DH], dt.bfloat16)
            wv_s = singles.tile([128, NC_ * DH], dt.bfloat16)
            wo_s = singles.tile([128, HL, D], dt.bfloat16)
            cosq_s = singles.tile([128, S], dt.bfloat16)
            sinq_s = singles.tile([128, S], dt.bfloat16)
            cosk_s = singles.tile([128, S], dt.bfloat16)
            sink_s = singles.tile([128, S], dt.bfloat16)
            tri_s = singles.tile([128, 128], dt.bfloat16)
            eps_s = singles.tile([128, 1], dt.float32)
            qT_s = singles.tile([128, HL, S], dt.bfloat16)
            kT_s = singles.tile([128, S], dt.bfloat16)
            v_s = singles.tile([128, NT, DH], dt.bfloat16)
            ctxT_s = singles.tile([128, HL, S], dt.bfloat16)

            nc.vector.memset(eps_s, EPS)

            # ---- input DMAs, in compute-consumption order per queue ----
            # Early xT chunks go out in 1024-col halves to halve the
            # per-chunk delivery latency while the K projection chases them;
            # wk is split fine-grained the same way.  Queue assignment keeps
            # each queue's k-th transfer ahead of PE's consumption of it.
            HB = 2 * BLK
            # gpsimd SWDGE: wk chunks 0-3 first (tiny), odd-late xT chunks
            nc.gpsimd.dma_start(out=wk_s[:, :DH], in_=wk[:, :DH])
            nc.gpsimd.dma_start(out=wk_s[:, DH : 2 * DH], in_=wk[:, DH : 2 * DH])
            nc.gpsimd.dma_start(out=wk_s[:, 2 * DH : 4 * DH], in_=wk[:, 2 * DH : 4 * DH])
            # sync (SP): chunk 0 split 512/1536 so block 0 lands first
            nc.sync.dma_start(out=xT_s[:, 0, :BLK], in_=xT[0][:, :BLK])
            nc.sync.dma_start(out=xT_s[:, 0, BLK:], in_=xT[0][:, BLK:])
            # scalar (ACT): xT1 halves first (the queue sits behind the
            # 1.3us act-table load anyway), xT3 halves, rest of wk
            nc.scalar.dma_start(out=xT_s[:, 1, :HB], in_=xT[1][:, :HB])
            nc.scalar.dma_start(out=xT_s[:, 1, HB:], in_=xT[1][:, HB:])
            nc.sync.dma_start(out=xT_s[:, 2, :HB], in_=xT[2][:, :HB])
            nc.sync.dma_start(out=xT_s[:, 2, HB:], in_=xT[2][:, HB:])
            nc.scalar.dma_start(out=xT_s[:, 3, :HB], in_=xT[3][:, :HB])
            nc.scalar.dma_start(out=xT_s[:, 3, HB:], in_=xT[3][:, HB:])
            nc.sync.dma_start(out=xT_s[:, 4, :HB], in_=xT[4][:, :HB])
            nc.sync.dma_start(out=xT_s[:, 4, HB:], in_=xT[4][:, HB:])
            nc.scalar.dma_start(out=wk_s[:, 4 * DH : 8 * DH], in_=wk[:, 4 * DH : 8 * DH])
            nc.scalar.dma_start(out=wk_s[:, 8 * DH :], in_=wk[:, 8 * DH :])
            nc.sync.dma_start(out=xT_s[:, 6, :HB], in_=xT[6][:, :HB])
            nc.sync.dma_start(out=xT_s[:, 6, HB:], in_=xT[6][:, HB:])
            for c in (8, 10, 12, 14):
                nc.sync.dma_start(out=xT_s[:, c, :], in_=xT[c])
            for c in (5, 7, 9, 11, 13, 15):
                nc.gpsimd.dma_start(out=xT_s[:, c, :], in_=xT[c])
            for c in range(6):
                nc.sync.dma_start(out=wq_s[:, c, :], in_=wq[c])
            for c in range(6, 11):
                nc.scalar.dma_start(out=wq_s[:, c, :], in_=wq[c])
            for c in range(11, NC_):
                nc.gpsimd.dma_start(out=wq_s[:, c, :], in_=wq[c])
            nc.scalar.dma_start(out=cosk_s[:], in_=cosk[:])
            nc.scalar.dma_start(out=sink_s[:], in_=sink[:])
            nc.scalar.dma_start(out=cosq_s[:], in_=cosq[:])
            nc.scalar.dma_start(out=sinq_s[:], in_=sinq[:])
            nc.gpsimd.dma_start(out=wv_s[:], in_=wv[:])
            nc.gpsimd.dma_start(out=tri_s[:], in_=tri[:])
            for h in range(HL):
                nc.gpsimd.dma_start(out=wo_s[:, h, :], in_=wo[h])

            Exp = mybir.ActivationFunctionType.Exp
            Sqrt = mybir.ActivationFunctionType.Sqrt
            Copy = mybir.ActivationFunctionType.Copy
            RADD = bass_isa.ReduceOp.add

            # ---- PE warm-up: dummy matmuls on a zeroed tile while the
            # first input DMAs are in flight, so the p-state ramp is burned
            # before real work arrives ----
            warm = work.tile([128, BLK], dt.bfloat16, tag="warm", bufs=1)
            weps = work.tile([128, 1], dt.bfloat16, tag="weps", bufs=1)
            nc.vector.memset(weps, 0.0)
            nc.vector.memset(warm, 0.0)
            ps_w = pmm.tile([1, BLK], dt.float32, tag="mm", name="ps_warm")
            for _ in range(2):
                nc.tensor.matmul(ps_w, weps, warm, start=True, stop=True)

            # ================= phase 1: projections (ACT: sqrt set) =========
            # K projection, chunk-outer into two 2-bank pair tiles: each
            # arriving xT chunk feeds 4 matmuls, PE chases the input DMAs.
            ps_kp = [
                ps2.tile([128, 2 * BLK], dt.float32, tag="s2", name=f"ps_kp{i}")
                for i in range(2)
            ]
            for c in range(NC_):
                for Bb in range(NB):
                    nc.tensor.matmul(
                        ps_kp[Bb // 2][:, (Bb % 2) * BLK : (Bb % 2) * BLK + BLK],
                        wk_s[:, ts(c, DH)], xT_s[:, c, ts(Bb, BLK)],
                        start=(c == 0), stop=(c == NC_ - 1),
                    )

            # Unified norm/rope pipeline units: 4 K blocks + 16 Q (h,Bb).
            # Stages: proj (PE, q only) -> prep (ACT raw copy [q], DVE
            # square, Pool rotate-half via 2 stream_shuffles) -> norm (Pool
            # all-reduce, ACT sqrt, DVE reciprocal) -> rope combine (DVE
            # t2=raw*cos, t1=rot*sin_signed, t12=t1+t2, dst=t12*rn_b).
            state = {}

            def k_raw(Bb):
                kraw = work.tile(
                    [128, BLK], dt.bfloat16, tag=f"kraw{Bb}", bufs=1,
                    name=f"kraw{Bb}",
                )
                nc.scalar.activation(
                    kraw, ps_kp[Bb // 2][:, (Bb % 2) * BLK : (Bb % 2) * BLK + BLK],
                    Copy,
                )
                state[("k", Bb)] = {"raw": kraw}

            def q_proj(i):
                Bb, h = divmod(i, HL)
                ps_q = pmm.tile([128, BLK], dt.float32, tag="mm")
                for c in range(NC_):
                    nc.tensor.matmul(
                        ps_q, wq_s[:, c, ts(h, DH)], xT_s[:, c, ts(Bb, BLK)],
                        start=(c == 0), stop=(c == NC_ - 1),
                    )
                state[("q", i)] = {"ps": ps_q}

            def u_prep(u):
                """raw copy (q only; k copies are pre-loop) + square + rot."""
                st = state[u]
                if u[0] == "q":
                    qraw = work.tile([128, BLK], dt.bfloat16, tag="qraw", bufs=3)
                    nc.scalar.activation(qraw, st.pop("ps"), Copy)
                    st["raw"] = qraw
                rot = work.tile([128, BLK], dt.bfloat16, tag="rot", bufs=2)
                nc.sync.dma_start(out=rot[:64, :], in_=st["raw"][64:, :])
                nc.sync.dma_start(out=rot[64:, :], in_=st["raw"][:64, :])
                st["rot"] = rot
                sq = work.tile([128, BLK], dt.bfloat16, tag="sq", bufs=2)
                nc.vector.tensor_mul(sq, st["raw"], st["raw"])
                st["sq"] = sq

            def u_norm(u):
                st = state[u]
                ss = work.tile([128, BLK], dt.float32, tag="ss", bufs=2)
                nc.gpsimd.partition_all_reduce(ss, st.pop("sq"), 128, RADD)
                rn_b = work.tile([128, BLK], dt.bfloat16, tag="rnb", bufs=2)
                nc.scalar.activation(rn_b, ss, Sqrt, scale=1.0 / DH, bias=eps_s)
                with nc.allow_low_precision(reason="rsqrt norm factor, bf16 ok"):
                    nc.vector.reciprocal(rn_b, rn_b)
                st["rn_b"] = rn_b

            def u_rope(u):
                st = state[u]
                if u[0] == "q":
                    Bb, h = divmod(u[1], HL)
                    cos_s, sin_s = cosq_s, sinq_s
                    dst = qT_s[:, h, ts(Bb, BLK)]
                else:
                    Bb = u[1]
                    cos_s, sin_s = cosk_s, sink_s
                    dst = kT_s[:, ts(Bb, BLK)]
                t2 = work.tile([128, BLK], dt.bfloat16, tag="t2", bufs=1)
                nc.vector.tensor_mul(t2, st["raw"], cos_s[:, ts(Bb, BLK)])
                t1 = work.tile([128, BLK], dt.bfloat16, tag="t1", bufs=1)
                nc.vector.tensor_mul(t1, st.pop("rot"), sin_s[:, ts(Bb, BLK)])
                t12 = work.tile([128, BLK], dt.bfloat16, tag="t12", bufs=1)
                nc.vector.tensor_add(t12, t2, t1)
                nc.vector.tensor_mul(dst, t12, st.pop("rn_b"))
                st.pop("raw", None)

            for Bb in range(NB):
                k_raw(Bb)
            units = []
            for j in range(NB):
                units.append(("k", j))
                units.append(("q", j))
            for i in range(NB, NB * HL):
                units.append(("q", i))

            for idx, u in enumerate(units):
                if u[0] == "q":
                    q_proj(u[1])
                if idx >= 1:
                    u_prep(units[idx - 1])
                if idx >= 2:
                    u_norm(units[idx - 2])
                    u_rope(units[idx - 2])
            u_prep(units[-1])
            u_norm(units[-2])
            u_rope(units[-2])
            u_norm(units[-1])
            u_rope(units[-1])

            # ---- V projection tiles: emitters used as phase-2 PE filler ----
            def make_vproj(t):
                def emit():
                    ps_v = pmm.tile([128, DH], dt.float32, tag="mm", name="ps_v")
                    for c in range(NC_):
                        nc.tensor.matmul(
                            ps_v, xT_s[:, c, ts(t, 128)], wv_s[:, ts(c, DH)],
                            start=(c == 0), stop=(c == NC_ - 1),
                        )
                    nc.vector.tensor_copy(v_s[:, t, :], ps_v)
                return emit

            # ================= phase 2: attention + out-proj (ACT: exp) =====
            osb_n = 0

            def attention(h, Bb, pre_pe=(), fillers=(), late_den=None):
                """sk-tile pairs: 2 score matmuls into one 2-bank PSUM tile,
                one scale-free exp per clean pair (per-half for staircase
                pairs), ctx matmuls one pair behind. pre_pe lands after pair
                0's matmuls; fillers dropped one per pair from pair 1 on,
                leftovers before the final ctx pair."""
                nonlocal osb_n
                ntile = 4 * Bb + 4
                npair = ntile // 2
                ps_ctx = pctx.tile([128, BLK], dt.float32, tag="ctx")
                acc = accp.tile([128, 2 * BLK], dt.bfloat16, tag="sumacc")
                exps = {}
                j0s = {}
                fill = list(fillers)
                p_drop = 2 if npair > 2 else 1
                fdrop = max(p_drop, npair // 2)

                def emit_ctx_pair(p):
                    for half in (0, 1):
                        t = 2 * p + half
                        nc.tensor.matmul(
                            ps_ctx[:, j0s[t]:], v_s[:, t, :],
                            exps[p][:, half * BLK + j0s[t] : (half + 1) * BLK],
                            start=(t == 0), stop=(t == ntile - 1),
                        )

                for p in range(npair):
                    ta, tb = 2 * p, 2 * p + 1
                    ja = max(0, ta * 128 - Bb * BLK)
                    jb = max(0, tb * 128 - Bb * BLK)
                    diag_a, diag_b = ta * 128 >= Bb * BLK, tb * 128 >= Bb * BLK
                    ps_S = ps2.tile([128, 2 * BLK], dt.float32, tag="s2", name="ps_S")
                    single = ja == 0 and not diag_b
                    nc.tensor.matmul(
                        ps_S[:, ja:BLK],
                        kT_s[:, ts(ta, 128)],
                        qT_s[:, h, Bb * BLK + ja : (Bb + 1) * BLK],
                        start=True, stop=True,
                    )
                    nc.tensor.matmul(
                        ps_S[:, BLK + jb : 2 * BLK],
                        kT_s[:, ts(tb, 128)],
                        qT_s[:, h, Bb * BLK + jb : (Bb + 1) * BLK],
                        start=True, stop=True,
                    )
                    if p == p_drop:
                        for f in pre_pe:
                            f()
                    if p >= fdrop and fill:
                        fill.pop(0)()
                    # exp first (post-norm scores are O(sqrt(dh)), so the
                    # masked region stays finite); zero masked columns on
                    # Pool afterwards (GPSIMD cannot touch PSUM, but expS
                    # lives in SBUF)
                    expS = expp.tile([128, 2 * BLK], dt.bfloat16, tag="exp")
                    if single:
                        nc.scalar.activation(expS[:], ps_S[:], Exp)
                    else:
                        nc.scalar.activation(expS[:, ja:BLK], ps_S[:, ja:BLK], Exp)
                        nc.scalar.activation(
                            expS[:, BLK + jb :], ps_S[:, BLK + jb :], Exp
                        )
                    if diag_a:
                        nc.gpsimd.tensor_mul(
                            expS[:, ja : ja + 128], expS[:, ja : ja + 128], tri_s
                        )
                    if diag_b:
                        nc.gpsimd.tensor_mul(
                            expS[:, BLK + jb : BLK + jb + 128],
                            expS[:, BLK + jb : BLK + jb + 128], tri_s,
                        )
                    if single:
                        if p == 0:
                            nc.vector.tensor_copy(acc, expS)
                        elif Bb == 3 and p <= 2:
                            # B3 units are DVE-tight; Pool has slack
                            nc.gpsimd.tensor_add(acc, acc, expS)
                        else:
                            nc.vector.tensor_add(acc, acc, expS)
                    elif p == 0:
                        # B0's first pair: init acc per-half, zero the gap
                        nc.vector.tensor_copy(acc[:, :BLK], expS[:, :BLK])
                        nc.gpsimd.memset(acc[:, BLK : BLK + jb], 0.0)
                        nc.vector.tensor_copy(
                            acc[:, BLK + jb :], expS[:, BLK + jb :]
                        )
                    else:
                        nc.vector.tensor_add(
                            acc[:, ja:BLK], acc[:, ja:BLK], expS[:, ja:BLK]
                        )
                        nc.vector.tensor_add(
                            acc[:, BLK + jb :], acc[:, BLK + jb :],
                            expS[:, BLK + jb :],
                        )
                    exps[p] = expS
                    j0s[ta] = ja
                    j0s[tb] = jb
                    if p >= 2:
                        emit_ctx_pair(p - 2)
                if late_den is not None:
                    late_den(acc)
                for f in fill:
                    f()
                emit_ctx_pair(npair - 2)
                emit_ctx_pair(npair - 1)
                return ps_ctx, acc

            def make_den_sum(acc, holder):
                def emit():
                    acc2 = work.tile([128, BLK], dt.bfloat16, tag="acc2", bufs=1)
                    nc.gpsimd.tensor_add(acc2, acc[:, :BLK], acc[:, BLK:])
                    den = work.tile([128, BLK], dt.float32, tag="den", bufs=1)
                    nc.gpsimd.partition_all_reduce(den, acc2, 128, RADD)
                    recip_b = work.tile([128, BLK], dt.bfloat16, tag="recipb")
                    with nc.allow_low_precision(reason="softmax denom, bf16 ok"):
                        nc.vector.reciprocal(recip_b, den)
                    holder["r"] = recip_b
                return emit

            def make_den_mul(h, Bb, ps_ctx, holder):
                def emit():
                    nc.vector.tensor_mul(
                        ctxT_s[:, h, ts(Bb, BLK)], ps_ctx, holder["r"]
                    )
                return emit

            def make_outproj(st, split=False, dve_only=False, act_only=False,
                             fine_tail=False):
                def emit():
                    nonlocal osb_n
                    osb = outp.tile([128, D], dt.bfloat16, tag="osb")
                    chunks = [(oc * BLK, BLK) for oc in range(4)]
                    if fine_tail:
                        chunks = chunks[:3] + [(3 * BLK, 448), (3 * BLK + 448, 64)]
                    for j0, w in chunks:
                        ps_o = pmm.tile([128, BLK], dt.float32, tag="mm", name="ps_o")
                        for h in range(HL):
                            nc.tensor.matmul(
                                ps_o[:, :w], ctxT_s[:, h, ts(st, 128)],
                                wo_s[:, h, j0 : j0 + w],
                                start=(h == 0), stop=(h == HL - 1),
                            )
                        if act_only:
                            nc.scalar.activation(osb[:, j0 : j0 + w], ps_o[:, :w], Copy)
                        elif dve_only or osb_n % 2 == 0:
                            nc.vector.tensor_copy(osb[:, j0 : j0 + w], ps_o[:, :w])
                        else:
                            nc.scalar.activation(osb[:, j0 : j0 + w], ps_o[:, :w], Copy)
                        osb_n += 1
                        if split:
                            nc.sync.dma_start(
                                out=out[ts(st, 128), j0 : j0 + w],
                                in_=osb[:, j0 : j0 + w],
                            )
                    if not split:
                        nc.sync.dma_start(out=out[ts(st, 128), :], in_=osb)
                return emit

            # filler schedule: V tiles into B0/B1 units, out-proj stripes
            # one block behind, last blocks' stripes in the tail
            order = [(Bb, h) for Bb in range(NB) for h in range(HL)]
            fillers = {
                (0, 0): [make_vproj(0), make_vproj(1), make_vproj(2), make_vproj(3)],
                (0, 1): [make_vproj(4), make_vproj(5)],
                (0, 2): [make_vproj(6), make_vproj(7)],
                (0, 3): [make_vproj(8), make_vproj(9)],
                (1, 0): [make_vproj(10), make_vproj(11)],
                (1, 1): [make_vproj(12), make_vproj(13)],
                (1, 2): [make_vproj(14), make_vproj(15)],
                (1, 3): [make_outproj(0)],
                (2, 0): [make_outproj(1)],
                (2, 1): [make_outproj(2)],
                (2, 2): [make_outproj(3)],
                (2, 3): [make_outproj(4)],
                (3, 0): [make_outproj(5, dve_only=True), make_outproj(6, dve_only=True)],
                (3, 1): [make_outproj(7, dve_only=True), make_outproj(8, dve_only=True)],
                (3, 2): [make_outproj(9, dve_only=True)],
                (3, 3): [],
            }

            # last unit: the denominator sum (acc halves + all-reduce +
            # reciprocal) is emitted inside the unit right after its last
            # exp-sum so the chain overlaps the final ctx pairs; the ctxT
            # multiply lands right after, before the tail's stripe matmuls
            # need it
            last = order[-1]
            pend_den = []
            for Bb, h in order:
                hold = {}
                late = None
                if (Bb, h) == last:
                    late = lambda a, hd=hold: make_den_sum(a, hd)()
                ps_ctx, acc = attention(
                    h, Bb, pre_pe=pend_den, fillers=fillers[(Bb, h)],
                    late_den=late,
                )
                if (Bb, h) == last:
                    pend_den = [make_den_mul(h, Bb, ps_ctx, hold)]
                else:
                    pend_den = [
                        make_den_sum(acc, hold), make_den_mul(h, Bb, ps_ctx, hold)
                    ]
            for f in pend_den:
                f()
            for st2 in (10, 11, 12, 13, 14):
                make_outproj(st2)()
            make_outproj(15, split=True, fine_tail=True)()
    nc.finalize()
    return nc


def _host_inputs(x, cos, sin, Wq, Wk, Wv, Wo, qn_w, kn_w):
    """Build the 8 per-core input maps (host-side sharding + layout prep)."""
    scale = DH ** -0.5
    qn_rot = np.concatenate([qn_w[64:], qn_w[:64]])
    kn_rot = np.concatenate([kn_w[64:], kn_w[:64]])
    # rotate-half sign: rows 0..63 of the rotated tensor carry -x2, so the
    # sin tables are negated on those partitions (rotation itself is an
    # unsigned 64-partition swap on Pool)
    sgn = np.ones((128, 1), dtype=np.float32)
    sgn[:64] = -1.0
    cosq = (cos.T * qn_w[:, None] * scale).astype(BF16)
    sinq = (sin.T * qn_rot[:, None] * scale * sgn).astype(BF16)
    cosk = (cos.T * kn_w[:, None]).astype(BF16)
    sink = (sin.T * kn_rot[:, None] * sgn).astype(BF16)
    ii = np.arange(128)
    tri = np.where(ii[None, :] < ii[:, None], 0.0, 1.0).astype(BF16)

    def cmajor(w):  # [2048, 128] -> [128, 16*128] (chunk-major free dim)
        return np.ascontiguousarray(
            w.reshape(NC_, 128, DH).transpose(1, 0, 2).reshape(128, NC_ * DH)
        ).astype(BF16)

    in_maps = []
    for core in range(8):
        b, g = core // 4, core % 4
        xTb = np.ascontiguousarray(x[b].T).astype(BF16).reshape(NC_, 128, S)
        in_maps.append({
            "xT": xTb,
            "wq": Wq[:, g * 512 : (g + 1) * 512].astype(BF16).reshape(NC_, 128, 512),
            "wk": cmajor(Wk[:, g * 128 : (g + 1) * 128]),
            "wv": cmajor(Wv[:, g * 128 : (g + 1) * 128]),
            "wo": Wo[g * 512 : (g + 1) * 512, :].astype(BF16).reshape(HL, 128, D),
            "cosq": cosq, "sinq": sinq, "cosk": cosk, "sink": sink,
            "tri": tri,
        })
    return in_maps


def kernel(x, mask, cos, sin, Wq, Wk, Wv, Wo, qn_w, kn_w, _trace=False):
    global _CACHED_NC
    x = np.asarray(x, dtype=np.float32)
    cos = np.asarray(cos, dtype=np.float32)
    sin = np.asarray(sin, dtype=np.float32)
    Wq = np.asarray(Wq, dtype=np.float32)
    Wk = np.asarray(Wk, dtype=np.float32)
    Wv = np.asarray(Wv, dtype=np.float32)
    Wo = np.asarray(Wo, dtype=np.float32)
    qn_w = np.asarray(qn_w, dtype=np.float32)
    kn_w = np.asarray(kn_w, dtype=np.float32)

    if _CACHED_NC is None:
        _CACHED_NC = build_nc()
    nc = _CACHED_NC
    in_maps = _host_inputs(x, cos, sin, Wq, Wk, Wv, Wo, qn_w, kn_w)
    res = run_bass_kernel_spmd(nc, in_maps, list(range(8)), trace=_trace)
    out = np.zeros((B, S, D), dtype=np.float32)
    for core in range(8):
        b = core // 4
        out[b] += np.asarray(res.results[core]["out"], dtype=np.float32)
    if _trace:
        return out, res
    return out
